# revision 2
# baseline (speedup 1.0000x reference)
"""EGNN (4-layer) Trainium2 kernel v2, 8 NeuronCores, edge-sharded.

Differences from v1:
 - P'[row] per-edge values come from a one-hot matmul (P_win stationary,
   host-precomputed indT moving) accumulated straight into the edge-MLP
   PSUM — no P dma_gather.
 - edge_attr and the radial cross-term are host-packed into one [40, EPAD]
   moving operand (eat1) so the first edge matmul is a single K=40 stream.
 - The agg indicator (ind) is built in ONE DVE is_equal per window via
   stride-0 broadcast APs.
 - m2's silu runs on [128,512] PSUM groups (4 tiles batched per ACT);
   m1's silu is one ACT per window.
 - Q tables are computed first each layer so the AllGather overlaps the
   P-table build.
 - Only the Q table is gathered per edge (SWDGE dma_gather).
"""

import math
import os
import sys
from contextlib import ExitStack

import numpy as np

sys.path.insert(0, "/opt/trn_rl_repo")

import ml_dtypes  # noqa: E402

BF = ml_dtypes.bfloat16

N = 20000
NCORES = 8
NPC = 2500          # nodes per core
NPCP = 2560         # padded nodes per core
WINS = 20           # node windows of 128 per core
H = 128
DE = 32
DK = 40             # packed edge-feature rows (32 ea + 3 radial + 5 pad)
L = 4
EPS = 1e-5

_CACHE = {}


def _groups(T):
    out = []
    left = T
    while left > 0:
        g = min(left, 4)
        out.append(g * 128)
        left -= g
    return out


GATHER_HBM = True   # gather Q from DRAM (faster SWDGE path than SBUF source)
USE_CNT = False     # runtime gather counts crash the ucode; keep static
SPLIT_AG = False    # Shared DRAM tiles allow only one writing instruction
AGW = 16            # windows 0..AGW-1 would go in the early AllGather


def _build(T, flags, wmax=None):
    """wmax: per-window gather token counts (max over cores), len WINS."""
    import concourse.bacc as bacc
    import concourse.tile as tile
    from concourse import mybir

    eb1_nz, eb2_nz, nb1_nz, nb2_nz, lnb_nz = flags
    EW = T * 128
    EPAD = WINS * EW
    GRPS = _groups(T)
    if wmax is None:
        wmax = (EW,) * WINS

    f32 = mybir.dt.float32
    bf16 = mybir.dt.bfloat16
    i16 = mybir.dt.int16
    i32 = mybir.dt.int32
    AX = mybir.AxisListType.X
    OP = mybir.AluOpType
    AF = mybir.ActivationFunctionType

    nc = bacc.Bacc(
        "TRN2",
        target_bir_lowering=False,
        debug=False,
        enable_asserts=False,
        num_devices=NCORES,
    )

    def din(name, shape, dt):
        return nc.dram_tensor(name, list(shape), dt, kind="ExternalInput").ap()

    h_d = din("h", (NPCP, H), f32)
    coords_d = din("coords", (NPCP, 4), f32)
    eat1_d = din("eat1", (DK, EPAD), bf16)
    indT_d = din("indT", (128, EPAD), bf16)
    idxq_d = din("idxq", (128, EPAD // 16), i16)
    wcnt_d = din("wcnt", (1, WINS), i32)
    rowrel_d = din("rowrel", (128, EPAD // 128), bf16)
    w1h_d = din("w1h", (L, H, H), bf16)
    w1c_d = din("w1c", (L, H, H), bf16)
    w1e_d = din("w1e40", (L, DK, H), bf16)
    wrb_d = din("wrb", (L, H, H), f32)
    ew2_d = din("ew2", (L, H, H), bf16)
    nw1h_d = din("nw1h", (L, H, H), bf16)
    nw1a_d = din("nw1a", (L, H, H), bf16)
    nw2_d = din("nw2", (L, H, H), bf16)
    eb1_d = din("eb1T", (H, L), f32)
    nb1_d = din("nb1T", (H, L), f32)
    nb2_d = din("nb2T", (H, L), f32)
    eb2b_d = din("eb2b", (L, H, H), f32) if eb2_nz else None
    lng_d = din("lngb", (H, H), f32)
    lnb_d = din("lnbb", (H, H), f32) if lnb_nz else None
    iota_d = din("iota", (H, H), bf16)
    idb_d = din("identb", (H, H), bf16)
    idf_d = din("identf", (H, H), f32)

    out_d = nc.dram_tensor("out", [NPCP, H], f32, kind="ExternalOutput").ap()

    with ExitStack() as ctx:
        tc = ctx.enter_context(tile.TileContext(nc))
        const = ctx.enter_context(tc.tile_pool(name="const", bufs=1))
        resid = ctx.enter_context(tc.tile_pool(name="resid", bufs=1))
        dram = ctx.enter_context(tc.tile_pool(name="dram", bufs=1, space="DRAM"))
        gpool = ctx.enter_context(tc.tile_pool(name="gpool", bufs=2))
        wpool = ctx.enter_context(tc.tile_pool(name="wpool", bufs=2))
        work = ctx.enter_context(tc.tile_pool(name="work", bufs=3))
        # PSUM: ps512 (edge pre1) 2 banks, psm2 (m2 batches) 2 banks,
        # psagg 2 banks, psm (tables/node/misc) 2 banks = 8
        ps512 = ctx.enter_context(tc.tile_pool(name="ps512", bufs=2, space="PSUM"))
        psm2 = ctx.enter_context(tc.tile_pool(name="psm2", bufs=2, space="PSUM"))
        psagg = ctx.enter_context(tc.tile_pool(name="psagg", bufs=1, space="PSUM"))
        psm = ctx.enter_context(tc.tile_pool(name="psm", bufs=3, space="PSUM"))

        sync = nc.sync

        # ---------- constants ----------
        iota_sb = const.tile([H, 1, H], bf16)
        sync.dma_start(out=iota_sb[:, 0, :], in_=iota_d[0:H, :])
        idb_sb = const.tile([H, H], bf16)
        sync.dma_start(out=idb_sb[:], in_=idb_d[:])
        idf_sb = const.tile([H, H], f32)
        sync.dma_start(out=idf_sb[:], in_=idf_d[:])
        lng_sb = const.tile([H, H], f32)
        sync.dma_start(out=lng_sb[:], in_=lng_d[:])
        lnb_sb = None
        if lnb_nz:
            lnb_sb = const.tile([H, H], f32)
            sync.dma_start(out=lnb_sb[:], in_=lnb_d[:])
        eb1_sb = const.tile([H, L], f32)
        sync.dma_start(out=eb1_sb[:], in_=eb1_d[:])
        nb1_sb = const.tile([H, L], f32)
        sync.dma_start(out=nb1_sb[:], in_=nb1_d[:])
        nb2_sb = const.tile([H, L], f32)
        sync.dma_start(out=nb2_sb[:], in_=nb2_d[:])

        def load_w(name, d, p, dt):
            t = const.tile([p, L, H], dt, name=name)
            sync.dma_start(out=t[:], in_=d.rearrange("l k f -> k l f"))
            return t

        w1h_sb = load_w("w1h_sb", w1h_d, H, bf16)
        w1c_sb = load_w("w1c_sb", w1c_d, H, bf16)
        w1e_sb = load_w("w1e_sb", w1e_d, DK, bf16)
        wrb_sb = load_w("wrb_sb", wrb_d, H, f32)
        ew2_sb = load_w("ew2_sb", ew2_d, H, bf16)
        nw1h_sb = load_w("nw1h_sb", nw1h_d, H, bf16)
        nw1a_sb = load_w("nw1a_sb", nw1a_d, H, bf16)
        nw2_sb = load_w("nw2_sb", nw2_d, H, bf16)
        eb2b_sb = load_w("eb2b_sb", eb2b_d, H, f32) if eb2_nz else None

        # ---------- resident state ----------
        hT = resid.tile([H, NPCP], bf16)
        zTa = resid.tile([H, NPCP], bf16)
        s_sb = resid.tile([H, WINS], f32)
        p_sb = resid.tile([H, WINS * H], bf16)
        idxq_sb = resid.tile([128, EPAD // 16], i16)
        sync.dma_start(out=idxq_sb[:], in_=idxq_d[:])
        wcnt_sb = resid.tile([1, WINS], i32)
        sync.dma_start(out=wcnt_sb[:], in_=wcnt_d[:])
        # per-window gather counts, loaded once (layer-independent)
        wcnt_regs = None
        if USE_CNT:
            wcnt_regs = [
                nc.gpsimd.value_load(
                    wcnt_sb[0:1, w : w + 1], min_val=16, max_val=EW
                )
                for w in range(WINS)
            ]
        rowrel_sb = resid.tile([128, EPAD // 128, 1], bf16)
        sync.dma_start(out=rowrel_sb[:, :, 0], in_=rowrel_d[:])

        # manual double-buffer for gathered Q; zero once so the never-written
        # tail columns ([wmax, EW)) stay finite (NaN x 0 would poison agg).
        gq_bufs = [resid.tile([128, 1, EW], bf16, name=f"gqb{i}") for i in range(2)]
        for g in gq_bufs:
            nc.vector.memset(g[:], 0.0)

        qown_dram = dram.tile([NPCP, H], bf16)

        # ---------- layer-0 setup: h^T, |c|^2 ----------
        for j in range(WINS):
            jsl = slice(j * H, (j + 1) * H)
            h_in = work.tile([H, H], f32, tag="h_in")
            sync.dma_start(out=h_in[:], in_=h_d[jsl, :])
            pst = psm.tile([H, H], f32, tag="pm")
            nc.tensor.transpose(pst[:], h_in[:], idf_sb[:])
            nc.vector.tensor_copy(hT[:, jsl], pst[:])

            c_in = work.tile([H, 4], f32, tag="c_in")
            sync.dma_start(out=c_in[:], in_=coords_d[jsl, :])
            csq = work.tile([H, 4], f32, tag="csq")
            nc.vector.tensor_mul(csq[:], c_in[:], c_in[:])
            nc.vector.reduce_sum(s_sb[:, j : j + 1], csq[:, 0:3], AX)

        # ---------- tables for a given layer, one 128-node chunk ----------
        def q_table_chunk(l, j):
            jsl = slice(j * H, (j + 1) * H)
            tfold = work.tile([H, H], f32, tag="tfold")
            nc.vector.tensor_scalar_mul(
                tfold[:], wrb_sb[:, l, :], s_sb[:, j : j + 1]
            )
            psq = psm.tile([H, H], f32, tag="pm")
            nc.tensor.matmul(
                psq[:], hT[:, jsl], w1c_sb[:, l, :], start=True, stop=True
            )
            qt = work.tile([H, H], bf16, tag="qt")
            nc.vector.tensor_tensor(qt[:], psq[:], tfold[:], OP.add)
            sync.dma_start(out=qown_dram[jsl, :], in_=qt[:])
            return tfold

        def p_table_chunk(l, j, tfold=None):
            jsl = slice(j * H, (j + 1) * H)
            if tfold is None:
                tfold = work.tile([H, H], f32, tag="tfold")
                nc.vector.tensor_scalar_mul(
                    tfold[:], wrb_sb[:, l, :], s_sb[:, j : j + 1]
                )
            psp = psm.tile([H, H], f32, tag="pm")
            nc.tensor.matmul(
                psp[:], hT[:, jsl], w1h_sb[:, l, :], start=True, stop=True
            )
            nc.vector.tensor_tensor(p_sb[:, jsl], psp[:], tfold[:], OP.add)

        qfull_drams = [
            dram.tile([NCORES * NPCP, H], bf16, addr_space="Shared", name=f"qfull_{l}")
            for l in range(L)
        ]

        NA = AGW * H                      # rows in the early shard
        NB = NPCP - NA                    # rows in the late shard
        RG = [list(range(NCORES))]

        def q_allgather(l, part):
            """part: None = whole, 'A' = windows [0, AGW), 'B' = the rest.
            Split output layout: qfull rows [0, 8*NA) hold the A shards
            (rank-major), rows [8*NA, ...) hold the B shards."""
            qf = qfull_drams[l]
            if part is None:
                nc.gpsimd.collective_compute(
                    "AllGather", mybir.AluOpType.bypass, replica_groups=RG,
                    ins=[qown_dram.opt()], outs=[qf.opt()],
                )
            elif part == "A":
                nc.gpsimd.collective_compute(
                    "AllGather", mybir.AluOpType.bypass, replica_groups=RG,
                    ins=[qown_dram[0:NA, :]], outs=[qf[0 : NCORES * NA, :]],
                )
            else:
                nc.gpsimd.collective_compute(
                    "AllGather", mybir.AluOpType.bypass, replica_groups=RG,
                    ins=[qown_dram[NA:NPCP, :]], outs=[qf[NCORES * NA :, :]],
                )

        def emit_allgathers(l, j):
            if not SPLIT_AG:
                if j == WINS - 1:
                    q_allgather(l, None)
            elif j == AGW - 1:
                q_allgather(l, "A")
            elif j == WINS - 1:
                q_allgather(l, "B")

        # initial tables (layer 0): Q first, collective, P under the collective
        for j in range(WINS):
            q_table_chunk(0, j)
            emit_allgathers(0, j)
        for j in range(WINS):
            p_table_chunk(0, j)

        # ---------- layers ----------
        for l in range(L):
            qfull_dram = qfull_drams[l]
            # edge pass
            for w in range(WINS):
                wsl = slice(w * EW, (w + 1) * EW)
                isl = slice(w * (EW // 16), (w + 1) * (EW // 16))
                jsl = slice(w * H, (w + 1) * H)

                gq = gq_bufs[w % 2]
                nw = int(wmax[w])
                isl = slice(w * (EW // 16), w * (EW // 16) + nw // 16)
                cnt = wcnt_regs[w] if USE_CNT else nw
                nc.gpsimd.dma_gather(
                    gq[:, :, :nw],
                    qfull_dram[:],
                    idxq_sb[:, isl],
                    nw,
                    cnt,
                    H,
                    transpose=True,
                    single_packet=False,
                )
                eaw = wpool.tile([DK, EW], bf16, tag="eaw")
                sync.dma_start(out=eaw[:], in_=eat1_d[:, wsl])
                itw = wpool.tile([128, EW], bf16, tag="itw")
                sync.dma_start(out=itw[:], in_=indT_d[:, wsl])

                # agg indicator for the whole window in one DVE op
                ind = gpool.tile([128, T, 128], bf16, tag="ind")
                nc.vector.tensor_tensor(
                    ind[:],
                    rowrel_sb[:, w * T : (w + 1) * T, :].to_broadcast([128, T, 128]),
                    iota_sb[:].to_broadcast([128, T, 128]),
                    OP.is_equal,
                )

                pre1 = work.tile([H, EW], bf16, tag="pre1")
                off = 0
                for gsz in GRPS:
                    gsl = slice(off, off + gsz)
                    ps1 = ps512.tile([H, 512], f32, tag="big")
                    nc.tensor.matmul(
                        ps1[:, :gsz], w1e_sb[:, l, :], eaw[:, gsl],
                        start=True, stop=False,
                    )
                    nc.tensor.matmul(
                        ps1[:, :gsz], p_sb[:, jsl], itw[:, gsl],
                        start=False, stop=True,
                    )
                    nc.vector.tensor_tensor(
                        pre1[:, gsl], ps1[:, :gsz], gq[:, 0, gsl], OP.add
                    )
                    off += gsz

                bias1 = eb1_sb[:, l : l + 1] if eb1_nz else 0.0
                m1t = work.tile([H, EW], bf16, tag="m1t")
                nc.scalar.activation(m1t[:], pre1[:], AF.Silu, bias=bias1)

                pagg = psagg.tile([H, H], f32, tag="pagg")
                gtile = 0
                off = 0
                for gsz in GRPS:
                    ntile = gsz // 128
                    pm2 = psm2.tile([H, 512], f32, tag="pm2")
                    for t in range(ntile):
                        tsl = slice(off + t * 128, off + (t + 1) * 128)
                        nc.tensor.matmul(
                            pm2[:, t * 128 : (t + 1) * 128],
                            m1t[:, tsl], ew2_sb[:, l, :],
                            start=True, stop=True,
                        )
                    m2s = work.tile([H, 512], bf16, tag="m2s")
                    if eb2_nz:
                        tm2 = work.tile([H, 512], f32, tag="tm2")
                        for t in range(ntile):
                            nc.vector.tensor_tensor(
                                tm2[:, t * 128 : (t + 1) * 128],
                                pm2[:, t * 128 : (t + 1) * 128],
                                eb2b_sb[:, l, :], OP.add,
                            )
                        nc.scalar.activation(
                            m2s[:, :gsz], tm2[:, :gsz], AF.Silu
                        )
                    else:
                        nc.scalar.activation(m2s[:, :gsz], pm2[:, :gsz], AF.Silu)
                    for t in range(ntile):
                        nc.tensor.matmul(
                            pagg[:],
                            m2s[:, t * 128 : (t + 1) * 128],
                            ind[:, gtile, :],
                            start=(gtile == 0), stop=(gtile == T - 1),
                        )
                        gtile += 1
                    off += gsz
                nc.vector.tensor_copy(zTa[:, w * H : (w + 1) * H], pagg[:])

                # tail for this 128-node chunk: node MLP (+ residual), then
                # either next layer's tables or the final LayerNorm. Emitted
                # inline so it runs while later windows wait on their gather.
                sl = slice(w * H, (w + 1) * H)
                psu = psm.tile([H, H], f32, tag="pm")
                nc.tensor.matmul(
                    psu[:], nw1h_sb[:, l, :], hT[:, sl], start=True, stop=False
                )
                nc.tensor.matmul(
                    psu[:], nw1a_sb[:, l, :], zTa[:, sl], start=False, stop=True
                )
                u = work.tile([H, H], bf16, tag="u")
                biasn = nb1_sb[:, l : l + 1] if nb1_nz else 0.0
                nc.scalar.activation(u[:], psu[:], AF.Silu, bias=biasn)
                pso = psm.tile([H, H], f32, tag="pm")
                nc.tensor.matmul(pso[:], nw2_sb[:, l, :], u[:], start=True, stop=True)
                if l == 0:
                    if nb2_nz:
                        nc.vector.tensor_scalar_add(
                            hT[:, sl], pso[:], nb2_sb[:, l : l + 1]
                        )
                    else:
                        nc.vector.tensor_copy(hT[:, sl], pso[:])
                else:
                    if nb2_nz:
                        nc.vector.scalar_tensor_tensor(
                            hT[:, sl], pso[:], nb2_sb[:, l : l + 1], hT[:, sl],
                            OP.add, OP.add,
                        )
                    else:
                        nc.vector.tensor_tensor(hT[:, sl], pso[:], hT[:, sl], OP.add)
                if l + 1 < L:
                    tf = q_table_chunk(l + 1, w)
                    emit_allgathers(l + 1, w)
                    p_table_chunk(l + 1, w, tf)
                else:
                    # final LayerNorm for this chunk
                    inv = 1.0 / H
                    pst = psm.tile([H, H], bf16, tag="pm")
                    nc.tensor.transpose(pst[:], hT[:, sl], idb_sb[:])
                    hn = work.tile([H, H], f32, tag="hn")
                    nc.vector.tensor_copy(hn[:], pst[:])
                    mu = work.tile([H, 1], f32, tag="mu")
                    nc.vector.reduce_sum(mu[:], hn[:], AX)
                    nc.vector.tensor_scalar_mul(mu[:], mu[:], inv)
                    xc = work.tile([H, H], f32, tag="xc")
                    nc.vector.tensor_scalar_sub(xc[:], hn[:], mu[:])
                    sq = work.tile([H, H], f32, tag="sq")
                    nc.vector.tensor_mul(sq[:], xc[:], xc[:])
                    var = work.tile([H, 1], f32, tag="var")
                    nc.vector.reduce_sum(var[:], sq[:], AX)
                    sd = work.tile([H, 1], f32, tag="sd")
                    nc.vector.tensor_scalar(sd[:], var[:], inv, EPS, OP.mult, OP.add)
                    nc.scalar.activation(
                        sd[:], sd[:], mybir.ActivationFunctionType.Sqrt
                    )
                    rstd = work.tile([H, 1], f32, tag="rstd")
                    nc.vector.reciprocal(rstd[:], sd[:])
                    on = work.tile([H, H], f32, tag="on")
                    nc.vector.tensor_scalar_mul(on[:], xc[:], rstd[:])
                    nc.vector.tensor_mul(on[:], on[:], lng_sb[:])
                    if lnb_nz:
                        nc.vector.tensor_add(on[:], on[:], lnb_sb[:])
                    sync.dma_start(out=out_d[sl, :], in_=on[:])

    nc.compile()
    return nc


def _wrap_idx(v):
    n = v.shape[0]
    t = v.reshape(n // 16, 16).T.astype(np.int16)
    return np.tile(t, (8, 1))


def _prep(inputs):
    """Host-side edge sort / packing. Returns (T, flags, shared, in_maps_core)."""
    h = np.asarray(inputs["h"], np.float32)
    coords = np.asarray(inputs["coords"], np.float32)
    edge_attr = np.asarray(inputs["edge_attr"], np.float32)
    edges = np.asarray(inputs["edges"]).astype(np.int64)
    ew1 = np.asarray(inputs["edge_w1"], np.float32)
    eb1 = np.asarray(inputs["edge_b1"], np.float32)
    ew2 = np.asarray(inputs["edge_w2"], np.float32)
    eb2 = np.asarray(inputs["edge_b2"], np.float32)
    nw1 = np.asarray(inputs["node_w1"], np.float32)
    nb1 = np.asarray(inputs["node_b1"], np.float32)
    nw2 = np.asarray(inputs["node_w2"], np.float32)
    nb2 = np.asarray(inputs["node_b2"], np.float32)
    ln_g = np.asarray(inputs["ln_g"], np.float32)
    ln_b = np.asarray(inputs["ln_b"], np.float32)

    E = edges.shape[1]
    row, col = edges[0], edges[1]

    owner = row // NPC
    gw = owner * WINS + (row - owner * NPC) // 128
    order = np.argsort(gw, kind="stable")
    counts = np.bincount(gw, minlength=NCORES * WINS)
    T = int(math.ceil(counts.max() / 128))
    EW = T * 128
    EPAD = WINS * EW

    gws = gw[order]
    starts = np.zeros(NCORES * WINS, np.int64)
    starts[1:] = np.cumsum(counts)[:-1]
    pos = np.arange(E) - starts[gws]
    slot = (gws % WINS) * EW + pos
    core = gws // WINS

    rowS = np.zeros((NCORES, EPAD), np.int64)
    colS = np.zeros((NCORES, EPAD), np.int64)
    valid = np.zeros((NCORES, EPAD), bool)
    eaS = np.zeros((NCORES, EPAD, DE), np.float32)
    rowS[core, slot] = row[order]
    colS[core, slot] = col[order]
    valid[core, slot] = True
    eaS[core, slot] = edge_attr[order]

    karr = np.arange(NCORES)[:, None]
    wloc = (np.arange(EPAD) // EW)[None, :]
    rowrel = np.where(valid, rowS - karr * NPC - 128 * wloc, -1).astype(np.float32)
    # token row in the gathered-Q DRAM table; with the split AllGather the
    # first AGW windows' shards are rank-major at the front, the rest after.
    ccore, cloc = colS // NPC, colS % NPC
    if SPLIT_AG:
        NA = AGW * 128
        idxq = np.where(
            cloc < NA,
            ccore * NA + cloc,
            NCORES * NA + ccore * (NPCP - NA) + (cloc - NA),
        )
    else:
        idxq = ccore * NPCP + cloc
    idxq = np.where(valid, idxq, 0).astype(np.int64)
    wcnt = np.maximum(counts.reshape(NCORES, WINS), 16).astype(np.int32)
    wmax = tuple(
        int(x)
        for x in np.minimum(
            (wcnt.max(axis=0) + 127) // 128 * 128, EW
        )
    )

    crS = np.where(valid[..., None], coords[rowS], 0.0).astype(np.float32)
    ccS = np.where(valid[..., None], coords[colS], 0.0).astype(np.float32)
    t1 = (crS * ccS).astype(BF)  # [NC, EPAD, 3]

    # indT: one-hot [node-in-window, slot] per core
    indT = np.zeros((NCORES, 128, EPAD), BF)
    kk, ss = np.nonzero(valid)
    indT[kk, rowrel[kk, ss].astype(np.int64), ss] = 1.0

    # eat1 packed [DK, EPAD]
    eat1 = np.zeros((NCORES, DK, EPAD), BF)
    eat1[:, 0:DE] = np.transpose(eaS, (0, 2, 1)).astype(BF)
    eat1[:, DE : DE + 3] = np.transpose(t1, (0, 2, 1))

    # weights
    w1h = ew1[:, 0:H, :]
    w1c = ew1[:, H : 2 * H, :]
    wr = ew1[:, 2 * H, :]
    w1e = ew1[:, 2 * H + 1 :, :]
    w1e40 = np.zeros((L, DK, H), np.float32)
    w1e40[:, 0:DE] = w1e
    w1e40[:, DE : DE + 3] = np.repeat((-2.0 * wr)[:, None, :], 3, axis=1)
    wrb = np.repeat(wr[:, None, :], H, axis=1).astype(np.float32)
    nw1h = nw1[:, :H, :]
    nw1a = nw1[:, H:, :]

    flags = (
        bool(np.any(eb1)), bool(np.any(eb2)),
        bool(np.any(nb1)), bool(np.any(nb2)), bool(np.any(ln_b)),
    )

    iota = np.tile(np.arange(H, dtype=np.float32), (H, 1))
    ident = np.eye(H, dtype=np.float32)

    shared = {
        "w1h": w1h.astype(BF), "w1c": w1c.astype(BF), "w1e40": w1e40.astype(BF),
        "wrb": wrb, "ew2": ew2.astype(BF),
        "nw1h": nw1h.astype(BF), "nw1a": nw1a.astype(BF), "nw2": nw2.astype(BF),
        "eb1T": np.ascontiguousarray(eb1.T), "nb1T": np.ascontiguousarray(nb1.T),
        "nb2T": np.ascontiguousarray(nb2.T),
        "lngb": np.tile(ln_g, (H, 1)).astype(np.float32),
        "iota": iota.astype(BF), "identb": ident.astype(BF), "identf": ident,
    }
    if flags[1]:
        shared["eb2b"] = np.repeat(eb2[:, None, :], H, axis=1).astype(np.float32)
    if flags[4]:
        shared["lnbb"] = np.tile(ln_b, (H, 1)).astype(np.float32)

    in_maps = []
    for k in range(NCORES):
        hk = np.zeros((NPCP, H), np.float32)
        hk[:NPC] = h[k * NPC : (k + 1) * NPC]
        ck = np.zeros((NPCP, 4), np.float32)
        ck[:NPC, :3] = coords[k * NPC : (k + 1) * NPC]
        m = {
            "h": hk,
            "coords": ck,
            "eat1": np.ascontiguousarray(eat1[k]),
            "indT": np.ascontiguousarray(indT[k]),
            "idxq": _wrap_idx(idxq[k]),
            "wcnt": wcnt[k : k + 1],
            "rowrel": np.ascontiguousarray(
                rowrel[k].reshape(EPAD // 128, 128).T
            ).astype(BF),
        }
        m.update(shared)
        in_maps.append(m)

    return T, flags, wmax, in_maps


def kernel(**inputs):
    from concourse.bass_utils import run_bass_kernel_spmd

    T, flags, wmax, in_maps = _prep(inputs)

    key = (T, flags, wmax)
    if key not in _CACHE:
        _CACHE[key] = _build(T, flags, wmax)
    nc = _CACHE[key]

    trace = bool(os.environ.get("EGNN_TRACE"))
    kw = {}
    if trace:
        kw = {"trace": True, "tmpdir": os.environ.get("EGNN_TRACE_DIR") or None}
    res = run_bass_kernel_spmd(nc, in_maps, list(range(NCORES)), **kw)
    if trace:
        print(f"HW exec time: {res.exec_time_ns} ns")
    outs = [res.results[k]["out"][:NPC] for k in range(NCORES)]
    return np.concatenate(outs, axis=0).astype(np.float32)


# revision 3
# speedup vs baseline: 1.0163x; 1.0163x over previous
"""EGNN (4-layer) Trainium2 kernel v2, 8 NeuronCores, edge-sharded.

Differences from v1:
 - P'[row] per-edge values come from a one-hot matmul (P_win stationary,
   host-precomputed indT moving) accumulated straight into the edge-MLP
   PSUM — no P dma_gather.
 - edge_attr and the radial cross-term are host-packed into one [40, EPAD]
   moving operand (eat1) so the first edge matmul is a single K=40 stream.
 - The agg indicator (ind) is built in ONE DVE is_equal per window via
   stride-0 broadcast APs.
 - m2's silu runs on [128,512] PSUM groups (4 tiles batched per ACT);
   m1's silu is one ACT per window.
 - Q tables are computed first each layer so the AllGather overlaps the
   P-table build.
 - Only the Q table is gathered per edge (SWDGE dma_gather).
"""

import math
import os
import sys
from contextlib import ExitStack

import numpy as np

sys.path.insert(0, "/opt/trn_rl_repo")

import ml_dtypes  # noqa: E402

BF = ml_dtypes.bfloat16

N = 20000
NCORES = 8
NPC = 2500          # nodes per core
NPCP = 2560         # padded nodes per core
WINS = 20           # node windows of 128 per core
H = 128
DE = 32
DK = 40             # packed edge-feature rows (32 ea + 3 radial + 5 pad)
L = 4
EPS = 1e-5

_CACHE = {}


def _groups(T):
    out = []
    left = T
    while left > 0:
        g = min(left, 4)
        out.append(g * 128)
        left -= g
    return out


GATHER_HBM = True   # gather Q from DRAM (faster SWDGE path than SBUF source)
USE_CNT = False     # runtime gather counts crash the ucode; keep static
SPLIT_AG = False    # Shared DRAM tiles allow only one writing instruction
AGW = 16            # windows 0..AGW-1 would go in the early AllGather


def _build(T, flags, wmax=None):
    """wmax: per-window gather token counts (max over cores), len WINS."""
    import concourse.bacc as bacc
    import concourse.tile as tile
    from concourse import mybir

    eb1_nz, eb2_nz, nb1_nz, nb2_nz, lnb_nz = flags
    EW = T * 128
    EPAD = WINS * EW
    GRPS = _groups(T)
    if wmax is None:
        wmax = (EW,) * WINS

    f32 = mybir.dt.float32
    bf16 = mybir.dt.bfloat16
    i16 = mybir.dt.int16
    i32 = mybir.dt.int32
    AX = mybir.AxisListType.X
    OP = mybir.AluOpType
    AF = mybir.ActivationFunctionType

    nc = bacc.Bacc(
        "TRN2",
        target_bir_lowering=False,
        debug=False,
        enable_asserts=False,
        num_devices=NCORES,
    )

    def din(name, shape, dt):
        return nc.dram_tensor(name, list(shape), dt, kind="ExternalInput").ap()

    h_d = din("h", (NPCP, H), f32)
    coords_d = din("coords", (NPCP, 4), f32)
    eat1_d = din("eat1", (DK, EPAD), bf16)
    indT_d = din("indT", (128, EPAD), bf16)
    idxq_d = din("idxq", (128, EPAD // 16), i16)
    wcnt_d = din("wcnt", (1, WINS), i32)
    rowrel_d = din("rowrel", (128, EPAD // 128), bf16)
    w1h_d = din("w1h", (L, H, H), bf16)
    w1c_d = din("w1c", (L, H, H), bf16)
    w1e_d = din("w1e40", (L, DK, H), bf16)
    wrb_d = din("wrb", (L, H, H), f32)
    ew2_d = din("ew2", (L, H, H), bf16)
    nw1h_d = din("nw1h", (L, H, H), bf16)
    nw1a_d = din("nw1a", (L, H, H), bf16)
    nw2_d = din("nw2", (L, H, H), bf16)
    eb1_d = din("eb1T", (H, L), f32)
    nb1_d = din("nb1T", (H, L), f32)
    nb2_d = din("nb2T", (H, L), f32)
    eb2b_d = din("eb2b", (L, H, H), f32) if eb2_nz else None
    lng_d = din("lngb", (H, H), f32)
    lnb_d = din("lnbb", (H, H), f32) if lnb_nz else None
    iota_d = din("iota", (H, H), bf16)
    idb_d = din("identb", (H, H), bf16)
    idf_d = din("identf", (H, H), f32)

    out_d = nc.dram_tensor("out", [NPCP, H], f32, kind="ExternalOutput").ap()

    with ExitStack() as ctx:
        tc = ctx.enter_context(tile.TileContext(nc))
        const = ctx.enter_context(tc.tile_pool(name="const", bufs=1))
        resid = ctx.enter_context(tc.tile_pool(name="resid", bufs=1))
        dram = ctx.enter_context(tc.tile_pool(name="dram", bufs=1, space="DRAM"))
        gpool = ctx.enter_context(tc.tile_pool(name="gpool", bufs=3))
        wpool = ctx.enter_context(tc.tile_pool(name="wpool", bufs=3))
        work = ctx.enter_context(tc.tile_pool(name="work", bufs=3))
        # PSUM: ps512 (edge pre1) 2 banks, psm2 (m2 batches) 2 banks,
        # psagg 2 banks, psm (tables/node/misc) 2 banks = 8
        ps512 = ctx.enter_context(tc.tile_pool(name="ps512", bufs=2, space="PSUM"))
        psm2 = ctx.enter_context(tc.tile_pool(name="psm2", bufs=2, space="PSUM"))
        psagg = ctx.enter_context(tc.tile_pool(name="psagg", bufs=1, space="PSUM"))
        psm = ctx.enter_context(tc.tile_pool(name="psm", bufs=3, space="PSUM"))

        sync = nc.sync

        # ---------- constants ----------
        iota_sb = const.tile([H, 1, H], bf16)
        sync.dma_start(out=iota_sb[:, 0, :], in_=iota_d[0:H, :])
        idb_sb = const.tile([H, H], bf16)
        sync.dma_start(out=idb_sb[:], in_=idb_d[:])
        idf_sb = const.tile([H, H], f32)
        sync.dma_start(out=idf_sb[:], in_=idf_d[:])
        lng_sb = const.tile([H, H], f32)
        sync.dma_start(out=lng_sb[:], in_=lng_d[:])
        lnb_sb = None
        if lnb_nz:
            lnb_sb = const.tile([H, H], f32)
            sync.dma_start(out=lnb_sb[:], in_=lnb_d[:])
        eb1_sb = const.tile([H, L], f32)
        sync.dma_start(out=eb1_sb[:], in_=eb1_d[:])
        nb1_sb = const.tile([H, L], f32)
        sync.dma_start(out=nb1_sb[:], in_=nb1_d[:])
        nb2_sb = const.tile([H, L], f32)
        sync.dma_start(out=nb2_sb[:], in_=nb2_d[:])

        def load_w(name, d, p, dt):
            t = const.tile([p, L, H], dt, name=name)
            sync.dma_start(out=t[:], in_=d.rearrange("l k f -> k l f"))
            return t

        w1h_sb = load_w("w1h_sb", w1h_d, H, bf16)
        w1c_sb = load_w("w1c_sb", w1c_d, H, bf16)
        w1e_sb = load_w("w1e_sb", w1e_d, DK, bf16)
        wrb_sb = load_w("wrb_sb", wrb_d, H, f32)
        ew2_sb = load_w("ew2_sb", ew2_d, H, bf16)
        nw1h_sb = load_w("nw1h_sb", nw1h_d, H, bf16)
        nw1a_sb = load_w("nw1a_sb", nw1a_d, H, bf16)
        nw2_sb = load_w("nw2_sb", nw2_d, H, bf16)
        eb2b_sb = load_w("eb2b_sb", eb2b_d, H, f32) if eb2_nz else None

        # ---------- resident state ----------
        hT = resid.tile([H, NPCP], bf16)
        zTa = resid.tile([H, NPCP], bf16)
        s_sb = resid.tile([H, WINS], f32)
        p_sb = resid.tile([H, WINS * H], bf16)
        idxq_sb = resid.tile([128, EPAD // 16], i16)
        sync.dma_start(out=idxq_sb[:], in_=idxq_d[:])
        wcnt_sb = resid.tile([1, WINS], i32)
        sync.dma_start(out=wcnt_sb[:], in_=wcnt_d[:])
        # per-window gather counts, loaded once (layer-independent)
        wcnt_regs = None
        if USE_CNT:
            wcnt_regs = [
                nc.gpsimd.value_load(
                    wcnt_sb[0:1, w : w + 1], min_val=16, max_val=EW
                )
                for w in range(WINS)
            ]
        rowrel_sb = resid.tile([128, EPAD // 128, 1], bf16)
        sync.dma_start(out=rowrel_sb[:, :, 0], in_=rowrel_d[:])

        # manual double-buffer for gathered Q; zero once so the never-written
        # tail columns ([wmax, EW)) stay finite (NaN x 0 would poison agg).
        gq_bufs = [resid.tile([128, 1, EW], bf16, name=f"gqb{i}") for i in range(3)]
        for g in gq_bufs:
            nc.vector.memset(g[:], 0.0)

        qown_dram = dram.tile([NPCP, H], bf16)

        # ---------- layer-0 setup: h^T, |c|^2 ----------
        for j in range(WINS):
            jsl = slice(j * H, (j + 1) * H)
            h_in = work.tile([H, H], f32, tag="h_in")
            sync.dma_start(out=h_in[:], in_=h_d[jsl, :])
            pst = psm.tile([H, H], f32, tag="pm")
            nc.tensor.transpose(pst[:], h_in[:], idf_sb[:])
            nc.vector.tensor_copy(hT[:, jsl], pst[:])

            c_in = work.tile([H, 4], f32, tag="c_in")
            sync.dma_start(out=c_in[:], in_=coords_d[jsl, :])
            csq = work.tile([H, 4], f32, tag="csq")
            nc.vector.tensor_mul(csq[:], c_in[:], c_in[:])
            nc.vector.reduce_sum(s_sb[:, j : j + 1], csq[:, 0:3], AX)

        # ---------- tables for a given layer, one 128-node chunk ----------
        def q_table_chunk(l, j):
            jsl = slice(j * H, (j + 1) * H)
            tfold = work.tile([H, H], f32, tag="tfold")
            nc.vector.tensor_scalar_mul(
                tfold[:], wrb_sb[:, l, :], s_sb[:, j : j + 1]
            )
            psq = psm.tile([H, H], f32, tag="pm")
            nc.tensor.matmul(
                psq[:], hT[:, jsl], w1c_sb[:, l, :], start=True, stop=True
            )
            qt = work.tile([H, H], bf16, tag="qt")
            nc.vector.tensor_tensor(qt[:], psq[:], tfold[:], OP.add)
            sync.dma_start(out=qown_dram[jsl, :], in_=qt[:])
            return tfold

        def p_table_chunk(l, j, tfold=None):
            jsl = slice(j * H, (j + 1) * H)
            if tfold is None:
                tfold = work.tile([H, H], f32, tag="tfold")
                nc.vector.tensor_scalar_mul(
                    tfold[:], wrb_sb[:, l, :], s_sb[:, j : j + 1]
                )
            psp = psm.tile([H, H], f32, tag="pm")
            nc.tensor.matmul(
                psp[:], hT[:, jsl], w1h_sb[:, l, :], start=True, stop=True
            )
            nc.vector.tensor_tensor(p_sb[:, jsl], psp[:], tfold[:], OP.add)

        qfull_drams = [
            dram.tile([NCORES * NPCP, H], bf16, addr_space="Shared", name=f"qfull_{l}")
            for l in range(L)
        ]

        NA = AGW * H                      # rows in the early shard
        NB = NPCP - NA                    # rows in the late shard
        RG = [list(range(NCORES))]

        def q_allgather(l, part):
            """part: None = whole, 'A' = windows [0, AGW), 'B' = the rest.
            Split output layout: qfull rows [0, 8*NA) hold the A shards
            (rank-major), rows [8*NA, ...) hold the B shards."""
            qf = qfull_drams[l]
            if part is None:
                nc.gpsimd.collective_compute(
                    "AllGather", mybir.AluOpType.bypass, replica_groups=RG,
                    ins=[qown_dram.opt()], outs=[qf.opt()],
                )
            elif part == "A":
                nc.gpsimd.collective_compute(
                    "AllGather", mybir.AluOpType.bypass, replica_groups=RG,
                    ins=[qown_dram[0:NA, :]], outs=[qf[0 : NCORES * NA, :]],
                )
            else:
                nc.gpsimd.collective_compute(
                    "AllGather", mybir.AluOpType.bypass, replica_groups=RG,
                    ins=[qown_dram[NA:NPCP, :]], outs=[qf[NCORES * NA :, :]],
                )

        def emit_allgathers(l, j):
            if not SPLIT_AG:
                if j == WINS - 1:
                    q_allgather(l, None)
            elif j == AGW - 1:
                q_allgather(l, "A")
            elif j == WINS - 1:
                q_allgather(l, "B")

        # initial tables (layer 0): Q first, collective, P under the collective
        for j in range(WINS):
            q_table_chunk(0, j)
            emit_allgathers(0, j)
        for j in range(WINS):
            p_table_chunk(0, j)

        # ---------- layers ----------
        for l in range(L):
            qfull_dram = qfull_drams[l]
            # edge pass
            for w in range(WINS):
                wsl = slice(w * EW, (w + 1) * EW)
                isl = slice(w * (EW // 16), (w + 1) * (EW // 16))
                jsl = slice(w * H, (w + 1) * H)

                gq = gq_bufs[w % 3]
                nw = int(wmax[w])
                isl = slice(w * (EW // 16), w * (EW // 16) + nw // 16)
                cnt = wcnt_regs[w] if USE_CNT else nw
                nc.gpsimd.dma_gather(
                    gq[:, :, :nw],
                    qfull_dram[:],
                    idxq_sb[:, isl],
                    nw,
                    cnt,
                    H,
                    transpose=True,
                    single_packet=False,
                )
                eaw = wpool.tile([DK, EW], bf16, tag="eaw")
                sync.dma_start(out=eaw[:], in_=eat1_d[:, wsl])
                itw = wpool.tile([128, EW], bf16, tag="itw")
                sync.dma_start(out=itw[:], in_=indT_d[:, wsl])

                # agg indicator for the whole window in one DVE op
                ind = gpool.tile([128, T, 128], bf16, tag="ind")
                nc.vector.tensor_tensor(
                    ind[:],
                    rowrel_sb[:, w * T : (w + 1) * T, :].to_broadcast([128, T, 128]),
                    iota_sb[:].to_broadcast([128, T, 128]),
                    OP.is_equal,
                )

                pre1 = work.tile([H, EW], bf16, tag="pre1")
                off = 0
                for gsz in GRPS:
                    gsl = slice(off, off + gsz)
                    ps1 = ps512.tile([H, 512], f32, tag="big")
                    nc.tensor.matmul(
                        ps1[:, :gsz], w1e_sb[:, l, :], eaw[:, gsl],
                        start=True, stop=False,
                    )
                    nc.tensor.matmul(
                        ps1[:, :gsz], p_sb[:, jsl], itw[:, gsl],
                        start=False, stop=True,
                    )
                    nc.vector.tensor_tensor(
                        pre1[:, gsl], ps1[:, :gsz], gq[:, 0, gsl], OP.add
                    )
                    off += gsz

                bias1 = eb1_sb[:, l : l + 1] if eb1_nz else 0.0
                m1t = work.tile([H, EW], bf16, tag="m1t")
                nc.scalar.activation(m1t[:], pre1[:], AF.Silu, bias=bias1)

                pagg = psagg.tile([H, H], f32, tag="pagg")
                gtile = 0
                off = 0
                for gsz in GRPS:
                    ntile = gsz // 128
                    pm2 = psm2.tile([H, 512], f32, tag="pm2")
                    for t in range(ntile):
                        tsl = slice(off + t * 128, off + (t + 1) * 128)
                        nc.tensor.matmul(
                            pm2[:, t * 128 : (t + 1) * 128],
                            m1t[:, tsl], ew2_sb[:, l, :],
                            start=True, stop=True,
                        )
                    m2s = work.tile([H, 512], bf16, tag="m2s")
                    if eb2_nz:
                        tm2 = work.tile([H, 512], f32, tag="tm2")
                        for t in range(ntile):
                            nc.vector.tensor_tensor(
                                tm2[:, t * 128 : (t + 1) * 128],
                                pm2[:, t * 128 : (t + 1) * 128],
                                eb2b_sb[:, l, :], OP.add,
                            )
                        nc.scalar.activation(
                            m2s[:, :gsz], tm2[:, :gsz], AF.Silu
                        )
                    else:
                        nc.scalar.activation(m2s[:, :gsz], pm2[:, :gsz], AF.Silu)
                    for t in range(ntile):
                        nc.tensor.matmul(
                            pagg[:],
                            m2s[:, t * 128 : (t + 1) * 128],
                            ind[:, gtile, :],
                            start=(gtile == 0), stop=(gtile == T - 1),
                        )
                        gtile += 1
                    off += gsz
                nc.vector.tensor_copy(zTa[:, w * H : (w + 1) * H], pagg[:])

                # tail for this 128-node chunk: node MLP (+ residual), then
                # either next layer's tables or the final LayerNorm. Emitted
                # inline so it runs while later windows wait on their gather.
                sl = slice(w * H, (w + 1) * H)
                psu = psm.tile([H, H], f32, tag="pm")
                nc.tensor.matmul(
                    psu[:], nw1h_sb[:, l, :], hT[:, sl], start=True, stop=False
                )
                nc.tensor.matmul(
                    psu[:], nw1a_sb[:, l, :], zTa[:, sl], start=False, stop=True
                )
                u = work.tile([H, H], bf16, tag="u")
                biasn = nb1_sb[:, l : l + 1] if nb1_nz else 0.0
                nc.scalar.activation(u[:], psu[:], AF.Silu, bias=biasn)
                pso = psm.tile([H, H], f32, tag="pm")
                nc.tensor.matmul(pso[:], nw2_sb[:, l, :], u[:], start=True, stop=True)
                if l == 0:
                    if nb2_nz:
                        nc.vector.tensor_scalar_add(
                            hT[:, sl], pso[:], nb2_sb[:, l : l + 1]
                        )
                    else:
                        nc.vector.tensor_copy(hT[:, sl], pso[:])
                else:
                    if nb2_nz:
                        nc.vector.scalar_tensor_tensor(
                            hT[:, sl], pso[:], nb2_sb[:, l : l + 1], hT[:, sl],
                            OP.add, OP.add,
                        )
                    else:
                        nc.vector.tensor_tensor(hT[:, sl], pso[:], hT[:, sl], OP.add)
                if l + 1 < L:
                    tf = q_table_chunk(l + 1, w)
                    emit_allgathers(l + 1, w)
                    p_table_chunk(l + 1, w, tf)
                else:
                    # final LayerNorm for this chunk
                    inv = 1.0 / H
                    pst = psm.tile([H, H], bf16, tag="pm")
                    nc.tensor.transpose(pst[:], hT[:, sl], idb_sb[:])
                    hn = work.tile([H, H], f32, tag="hn")
                    nc.vector.tensor_copy(hn[:], pst[:])
                    mu = work.tile([H, 1], f32, tag="mu")
                    nc.vector.reduce_sum(mu[:], hn[:], AX)
                    nc.vector.tensor_scalar_mul(mu[:], mu[:], inv)
                    xc = work.tile([H, H], f32, tag="xc")
                    nc.vector.tensor_scalar_sub(xc[:], hn[:], mu[:])
                    sq = work.tile([H, H], f32, tag="sq")
                    nc.vector.tensor_mul(sq[:], xc[:], xc[:])
                    var = work.tile([H, 1], f32, tag="var")
                    nc.vector.reduce_sum(var[:], sq[:], AX)
                    sd = work.tile([H, 1], f32, tag="sd")
                    nc.vector.tensor_scalar(sd[:], var[:], inv, EPS, OP.mult, OP.add)
                    nc.scalar.activation(
                        sd[:], sd[:], mybir.ActivationFunctionType.Sqrt
                    )
                    rstd = work.tile([H, 1], f32, tag="rstd")
                    nc.vector.reciprocal(rstd[:], sd[:])
                    on = work.tile([H, H], f32, tag="on")
                    nc.vector.tensor_scalar_mul(on[:], xc[:], rstd[:])
                    nc.vector.tensor_mul(on[:], on[:], lng_sb[:])
                    if lnb_nz:
                        nc.vector.tensor_add(on[:], on[:], lnb_sb[:])
                    sync.dma_start(out=out_d[sl, :], in_=on[:])

    nc.compile()
    return nc


def _wrap_idx(v):
    n = v.shape[0]
    t = v.reshape(n // 16, 16).T.astype(np.int16)
    return np.tile(t, (8, 1))


def _prep(inputs):
    """Host-side edge sort / packing. Returns (T, flags, shared, in_maps_core)."""
    h = np.asarray(inputs["h"], np.float32)
    coords = np.asarray(inputs["coords"], np.float32)
    edge_attr = np.asarray(inputs["edge_attr"], np.float32)
    edges = np.asarray(inputs["edges"]).astype(np.int64)
    ew1 = np.asarray(inputs["edge_w1"], np.float32)
    eb1 = np.asarray(inputs["edge_b1"], np.float32)
    ew2 = np.asarray(inputs["edge_w2"], np.float32)
    eb2 = np.asarray(inputs["edge_b2"], np.float32)
    nw1 = np.asarray(inputs["node_w1"], np.float32)
    nb1 = np.asarray(inputs["node_b1"], np.float32)
    nw2 = np.asarray(inputs["node_w2"], np.float32)
    nb2 = np.asarray(inputs["node_b2"], np.float32)
    ln_g = np.asarray(inputs["ln_g"], np.float32)
    ln_b = np.asarray(inputs["ln_b"], np.float32)

    E = edges.shape[1]
    row, col = edges[0], edges[1]

    owner = row // NPC
    gw = owner * WINS + (row - owner * NPC) // 128
    order = np.argsort(gw, kind="stable")
    counts = np.bincount(gw, minlength=NCORES * WINS)
    T = int(math.ceil(counts.max() / 128))
    EW = T * 128
    EPAD = WINS * EW

    gws = gw[order]
    starts = np.zeros(NCORES * WINS, np.int64)
    starts[1:] = np.cumsum(counts)[:-1]
    pos = np.arange(E) - starts[gws]
    slot = (gws % WINS) * EW + pos
    core = gws // WINS

    rowS = np.zeros((NCORES, EPAD), np.int64)
    colS = np.zeros((NCORES, EPAD), np.int64)
    valid = np.zeros((NCORES, EPAD), bool)
    eaS = np.zeros((NCORES, EPAD, DE), np.float32)
    rowS[core, slot] = row[order]
    colS[core, slot] = col[order]
    valid[core, slot] = True
    eaS[core, slot] = edge_attr[order]

    karr = np.arange(NCORES)[:, None]
    wloc = (np.arange(EPAD) // EW)[None, :]
    rowrel = np.where(valid, rowS - karr * NPC - 128 * wloc, -1).astype(np.float32)
    # token row in the gathered-Q DRAM table; with the split AllGather the
    # first AGW windows' shards are rank-major at the front, the rest after.
    ccore, cloc = colS // NPC, colS % NPC
    if SPLIT_AG:
        NA = AGW * 128
        idxq = np.where(
            cloc < NA,
            ccore * NA + cloc,
            NCORES * NA + ccore * (NPCP - NA) + (cloc - NA),
        )
    else:
        idxq = ccore * NPCP + cloc
    idxq = np.where(valid, idxq, 0).astype(np.int64)
    wcnt = np.maximum(counts.reshape(NCORES, WINS), 16).astype(np.int32)
    wmax = tuple(
        int(x)
        for x in np.minimum(
            (wcnt.max(axis=0) + 127) // 128 * 128, EW
        )
    )

    crS = np.where(valid[..., None], coords[rowS], 0.0).astype(np.float32)
    ccS = np.where(valid[..., None], coords[colS], 0.0).astype(np.float32)
    t1 = (crS * ccS).astype(BF)  # [NC, EPAD, 3]

    # indT: one-hot [node-in-window, slot] per core
    indT = np.zeros((NCORES, 128, EPAD), BF)
    kk, ss = np.nonzero(valid)
    indT[kk, rowrel[kk, ss].astype(np.int64), ss] = 1.0

    # eat1 packed [DK, EPAD]
    eat1 = np.zeros((NCORES, DK, EPAD), BF)
    eat1[:, 0:DE] = np.transpose(eaS, (0, 2, 1)).astype(BF)
    eat1[:, DE : DE + 3] = np.transpose(t1, (0, 2, 1))

    # weights
    w1h = ew1[:, 0:H, :]
    w1c = ew1[:, H : 2 * H, :]
    wr = ew1[:, 2 * H, :]
    w1e = ew1[:, 2 * H + 1 :, :]
    w1e40 = np.zeros((L, DK, H), np.float32)
    w1e40[:, 0:DE] = w1e
    w1e40[:, DE : DE + 3] = np.repeat((-2.0 * wr)[:, None, :], 3, axis=1)
    wrb = np.repeat(wr[:, None, :], H, axis=1).astype(np.float32)
    nw1h = nw1[:, :H, :]
    nw1a = nw1[:, H:, :]

    flags = (
        bool(np.any(eb1)), bool(np.any(eb2)),
        bool(np.any(nb1)), bool(np.any(nb2)), bool(np.any(ln_b)),
    )

    iota = np.tile(np.arange(H, dtype=np.float32), (H, 1))
    ident = np.eye(H, dtype=np.float32)

    shared = {
        "w1h": w1h.astype(BF), "w1c": w1c.astype(BF), "w1e40": w1e40.astype(BF),
        "wrb": wrb, "ew2": ew2.astype(BF),
        "nw1h": nw1h.astype(BF), "nw1a": nw1a.astype(BF), "nw2": nw2.astype(BF),
        "eb1T": np.ascontiguousarray(eb1.T), "nb1T": np.ascontiguousarray(nb1.T),
        "nb2T": np.ascontiguousarray(nb2.T),
        "lngb": np.tile(ln_g, (H, 1)).astype(np.float32),
        "iota": iota.astype(BF), "identb": ident.astype(BF), "identf": ident,
    }
    if flags[1]:
        shared["eb2b"] = np.repeat(eb2[:, None, :], H, axis=1).astype(np.float32)
    if flags[4]:
        shared["lnbb"] = np.tile(ln_b, (H, 1)).astype(np.float32)

    in_maps = []
    for k in range(NCORES):
        hk = np.zeros((NPCP, H), np.float32)
        hk[:NPC] = h[k * NPC : (k + 1) * NPC]
        ck = np.zeros((NPCP, 4), np.float32)
        ck[:NPC, :3] = coords[k * NPC : (k + 1) * NPC]
        m = {
            "h": hk,
            "coords": ck,
            "eat1": np.ascontiguousarray(eat1[k]),
            "indT": np.ascontiguousarray(indT[k]),
            "idxq": _wrap_idx(idxq[k]),
            "wcnt": wcnt[k : k + 1],
            "rowrel": np.ascontiguousarray(
                rowrel[k].reshape(EPAD // 128, 128).T
            ).astype(BF),
        }
        m.update(shared)
        in_maps.append(m)

    return T, flags, wmax, in_maps


def kernel(**inputs):
    from concourse.bass_utils import run_bass_kernel_spmd

    T, flags, wmax, in_maps = _prep(inputs)

    key = (T, flags, wmax)
    if key not in _CACHE:
        _CACHE[key] = _build(T, flags, wmax)
    nc = _CACHE[key]

    trace = bool(os.environ.get("EGNN_TRACE"))
    kw = {}
    if trace:
        kw = {"trace": True, "tmpdir": os.environ.get("EGNN_TRACE_DIR") or None}
    res = run_bass_kernel_spmd(nc, in_maps, list(range(NCORES)), **kw)
    if trace:
        print(f"HW exec time: {res.exec_time_ns} ns")
    outs = [res.results[k]["out"][:NPC] for k in range(NCORES)]
    return np.concatenate(outs, axis=0).astype(np.float32)
